# revision 1
# baseline (speedup 1.0000x reference)
"""Trainium2 Bass kernel for nn_DRUCell: 8-way data-parallel DRU cell.

reference:
    xh = concat([x, h], 1)                  # [B, IN+H]
    lin = xh @ W_in + b_in                  # [B, 2H]
    learn = tanh(lin[:, :H]); f = sigmoid(lin[:, H:])
    h_new = f * h + (1 - f) * learn
    out = tanh(concat([x, h_new], 1) @ W_out + b_out)
    returns (out, h_new)

Strategy: shard batch across the 8 NeuronCores (2048 rows each), replicate
weights. On-device everything lives feature-major ([feature, batch]) so the
TensorE contraction (over features) maps to partitions with no on-device
transposes; the host pre-transposes the shards (free relative to HW time) and
transposes the outputs back. Matmul operands run in bf16 (fp32 PSUM
accumulation); h is kept in fp32 for the elementwise h_new update.
"""

import numpy as np
import ml_dtypes
from contextlib import ExitStack

import concourse.bass as bass
import concourse.mybir as mybir
import concourse.tile as tile
from concourse import bacc
from concourse.bass_utils import run_bass_kernel_spmd

B, IN, H = 16384, 512, 512
NCORES = 8
BL = B // NCORES  # batch rows per core
P = 128
NB = 512          # batch columns per device tile
NT = BL // NB
KIN = IN // P     # x feature chunks
KH = H // P       # h feature chunks
K1 = KIN + KH     # contraction chunks for both matmuls
MO1 = 2 * H // P  # mm1 output chunks (learn 0..KH-1, forget KH..)
MO2 = H // P      # mm2 output chunks

MM_MODE = "bf16"  # "bf16" | "f32r" | "f32"

_nc_cache = {}


def _build(mm_mode):
    f32 = mybir.dt.float32
    bf16 = mybir.dt.bfloat16
    mm_dt = bf16 if mm_mode == "bf16" else f32

    def mm_ap(ap):
        return ap.bitcast(mybir.dt.float32r) if mm_mode == "f32r" else ap

    nc = bacc.Bacc("TRN2", target_bir_lowering=False, debug=False, num_devices=NCORES)

    xT_d = nc.dram_tensor("xT", [IN, BL], mm_dt, kind="ExternalInput")
    hT_d = nc.dram_tensor("hT", [H, BL], f32, kind="ExternalInput")
    w_in_d = nc.dram_tensor("w_in", [IN + H, 2 * H], mm_dt, kind="ExternalInput")
    w_out_d = nc.dram_tensor("w_out", [IN + H, H], mm_dt, kind="ExternalInput")
    b_in_d = nc.dram_tensor("b_in", [P, MO1], f32, kind="ExternalInput")
    b_out_d = nc.dram_tensor("b_out", [P, MO2], f32, kind="ExternalInput")
    if mm_mode == "bf16":
        hTc_d = nc.dram_tensor("hTc", [H, BL], bf16, kind="ExternalInput")
    h_newT_d = nc.dram_tensor("h_newT", [H, BL], f32, kind="ExternalOutput")
    outT_d = nc.dram_tensor("outT", [H, BL], f32, kind="ExternalOutput")

    AFT = mybir.ActivationFunctionType

    with tile.TileContext(nc) as tc, ExitStack() as ctx:
        cpool = ctx.enter_context(tc.tile_pool(name="consts", bufs=1))
        inpool = ctx.enter_context(tc.tile_pool(name="inputs", bufs=2))
        work = ctx.enter_context(tc.tile_pool(name="work", bufs=2))
        tmp_pool = ctx.enter_context(tc.tile_pool(name="tmp", bufs=4))
        psum1 = ctx.enter_context(tc.tile_pool(name="psum1", bufs=4, space="PSUM"))
        psum2 = ctx.enter_context(tc.tile_pool(name="psum2", bufs=2, space="PSUM"))

        # ---- resident constants: weights + biases ----
        w_in_sb = []
        for k in range(K1):
            wt = cpool.tile([P, 2 * H], mm_dt, name=f"w_in_{k}")
            nc.sync.dma_start(wt[:], w_in_d[k * P:(k + 1) * P, :])
            w_in_sb.append(wt)
        w_out_sb = []
        for k in range(K1):
            wt = cpool.tile([P, H], mm_dt, name=f"w_out_{k}")
            nc.sync.dma_start(wt[:], w_out_d[k * P:(k + 1) * P, :])
            w_out_sb.append(wt)
        b_in_sb = cpool.tile([P, MO1], f32, name="b_in_sb")
        nc.sync.dma_start(b_in_sb[:], b_in_d[:])
        b_out_sb = cpool.tile([P, MO2], f32, name="b_out_sb")
        nc.sync.dma_start(b_out_sb[:], b_out_d[:])

        # feature-major DRAM views for whole-block stores
        hn_dram = h_newT_d.ap().rearrange("(c p) n -> p c n", p=P)
        out_dram = outT_d.ap().rearrange("(c p) n -> p c n", p=P)

        for j in range(NT):
            bs = bass.ts(j, NB)

            # ---- stream this batch tile's activations ----
            xT_sb = []
            for k in range(KIN):
                t = inpool.tile([P, NB], mm_dt, name=f"xT_sb_{k}", tag=f"xT{k}")
                nc.sync.dma_start(t[:], xT_d[k * P:(k + 1) * P, bs])
                xT_sb.append(t)
            hT_sb = []
            for c in range(KH):
                t = inpool.tile([P, NB], f32, name=f"hT_sb_{c}", tag=f"hT{c}")
                nc.sync.dma_start(t[:], hT_d[c * P:(c + 1) * P, bs])
                hT_sb.append(t)
            if mm_mode == "bf16":
                hTc_sb = []
                for c in range(KH):
                    t = inpool.tile([P, NB], bf16, name=f"hTc_sb_{c}", tag=f"hTc{c}")
                    nc.sync.dma_start(t[:], hTc_d[c * P:(c + 1) * P, bs])
                    hTc_sb.append(t)
            else:
                hTc_sb = hT_sb

            learn = work.tile([P, KH * NB], f32, name="learn", tag="learn")
            forget = work.tile([P, KH * NB], f32, name="forget", tag="forget")
            hn = work.tile([P, KH * NB], f32, name="hn", tag="hn")
            if mm_mode == "bf16":
                hnc = work.tile([P, KH * NB], bf16, name="hnc", tag="hnc")

            # mm1 interleaved (learn_c, forget_c) so h_new chunk c's DVE work
            # overlaps the remaining matmuls instead of serializing at the end
            for c in range(KH):
                for mo in (c, c + KH):
                    ps = psum1.tile([P, NB], f32, name="ps1", tag="ps1")
                    for k in range(K1):
                        rhs_t = xT_sb[k] if k < KIN else hTc_sb[k - KIN]
                        nc.tensor.matmul(
                            ps[:],
                            mm_ap(w_in_sb[k][:, mo * P:(mo + 1) * P]),
                            mm_ap(rhs_t[:]),
                            start=(k == 0),
                            stop=(k == K1 - 1),
                        )
                    dst = (learn if mo < KH else forget)[:, c * NB:(c + 1) * NB]
                    nc.scalar.activation(
                        dst,
                        ps[:],
                        AFT.Tanh if mo < KH else AFT.Sigmoid,
                        bias=b_in_sb[:, mo:mo + 1],
                    )
                cs = bass.ts(c, NB)
                t = tmp_pool.tile([P, NB], f32, name="t", tag="t")
                nc.vector.tensor_sub(t[:], hT_sb[c][:], learn[:, cs])
                nc.vector.tensor_mul(t[:], t[:], forget[:, cs])
                nc.vector.tensor_add(hn[:, cs], t[:], learn[:, cs])
                if mm_mode == "bf16":
                    nc.vector.tensor_copy(hnc[:, cs], hn[:, cs])

            nc.sync.dma_start(
                hn_dram[:, :, bs], hn[:].rearrange("p (c n) -> p c n", c=KH)
            )

            hsrc = hnc if mm_mode == "bf16" else hn
            out_t = work.tile([P, MO2 * NB], f32, name="out_t", tag="out_t")
            for mo in range(MO2):
                ps = psum2.tile([P, NB], f32, name="ps2", tag="ps2")
                for k in range(K1):
                    if k < KIN:
                        rhs = xT_sb[k][:]
                    else:
                        rhs = hsrc[:, bass.ts(k - KIN, NB)]
                    nc.tensor.matmul(
                        ps[:],
                        mm_ap(w_out_sb[k][:, mo * P:(mo + 1) * P]),
                        mm_ap(rhs),
                        start=(k == 0),
                        stop=(k == K1 - 1),
                    )
                nc.scalar.activation(
                    out_t[:, bass.ts(mo, NB)],
                    ps[:],
                    AFT.Tanh,
                    bias=b_out_sb[:, mo:mo + 1],
                )
            nc.sync.dma_start(
                out_dram[:, :, bs], out_t[:].rearrange("p (c n) -> p c n", c=MO2)
            )

    nc.compile()
    return nc


def _get_nc(mm_mode):
    if mm_mode not in _nc_cache:
        _nc_cache[mm_mode] = _build(mm_mode)
    return _nc_cache[mm_mode]


def _run(x, h, W_in, b_in, W_out, b_out, mm_mode=MM_MODE, trace=False):
    x = np.asarray(x, dtype=np.float32)
    h = np.asarray(h, dtype=np.float32)
    W_in = np.asarray(W_in, dtype=np.float32)
    b_in = np.asarray(b_in, dtype=np.float32)
    W_out = np.asarray(W_out, dtype=np.float32)
    b_out = np.asarray(b_out, dtype=np.float32)

    bf16 = ml_dtypes.bfloat16
    mm_np = bf16 if mm_mode == "bf16" else np.float32
    w_in_m = W_in.astype(mm_np)
    w_out_m = W_out.astype(mm_np)
    b_in_m = np.ascontiguousarray(b_in.reshape(MO1, P).T)
    b_out_m = np.ascontiguousarray(b_out.reshape(MO2, P).T)

    in_maps = []
    for i in range(NCORES):
        sl = slice(i * BL, (i + 1) * BL)
        hT = np.ascontiguousarray(h[sl].T)
        m = {
            "xT": np.ascontiguousarray(x[sl].T).astype(mm_np),
            "hT": hT,
            "w_in": w_in_m,
            "w_out": w_out_m,
            "b_in": b_in_m,
            "b_out": b_out_m,
        }
        if mm_mode == "bf16":
            m["hTc"] = hT.astype(bf16)
        in_maps.append(m)

    nc = _get_nc(mm_mode)
    res = run_bass_kernel_spmd(nc, in_maps, list(range(NCORES)), trace=trace)

    out = np.empty((B, H), dtype=np.float32)
    h_new = np.empty((B, H), dtype=np.float32)
    for i in range(NCORES):
        sl = slice(i * BL, (i + 1) * BL)
        out[sl] = res.results[i]["outT"].T
        h_new[sl] = res.results[i]["h_newT"].T
    return (out, h_new), res


def kernel(x, h, W_in, b_in, W_out, b_out):
    (out, h_new), _ = _run(x, h, W_in, b_in, W_out, b_out)
    return (out, h_new)


# revision 4
# speedup vs baseline: 1.0954x; 1.0954x over previous
"""Trainium2 Bass kernel for nn_DRUCell: 8-way data-parallel DRU cell.

reference:
    xh = concat([x, h], 1)                  # [B, IN+H]
    lin = xh @ W_in + b_in                  # [B, 2H]
    learn = tanh(lin[:, :H]); f = sigmoid(lin[:, H:])
    h_new = f * h + (1 - f) * learn
    out = tanh(concat([x, h_new], 1) @ W_out + b_out)
    returns (out, h_new)

Strategy: shard batch across the 8 NeuronCores (2048 rows each), replicate
weights. On-device everything lives feature-major ([feature, batch]) so the
TensorE contraction (over features) maps to partitions with no on-device
transposes; the host pre-transposes the shards (free relative to HW time) and
transposes the outputs back. Matmul operands run in bf16 (fp32 PSUM
accumulation); h is kept in fp32 for the elementwise h_new update.

DMA issue order is tuned so the first batch tile's activations land before
the second half of the weights — the PE starts ~5us in instead of ~22us.
mm2 runs k-outer so the h_new bf16 cast chain is hidden behind the x-part
matmuls. Stores go through GpSimd (SWDGE) to keep SP free for loads.
"""

import numpy as np
import ml_dtypes
from contextlib import ExitStack

import concourse.bass as bass
import concourse.mybir as mybir
import concourse.tile as tile
from concourse import bacc
from concourse.bass_utils import run_bass_kernel_spmd

B, IN, H = 16384, 512, 512
NCORES = 8
BL = B // NCORES  # batch rows per core
P = 128
NB = 512          # batch columns per device tile
NT = BL // NB
KIN = IN // P     # x feature chunks
KH = H // P       # h feature chunks
K1 = KIN + KH     # contraction chunks for both matmuls
MO1 = 2 * H // P  # mm1 output chunks (learn 0..KH-1, forget KH..)
MO2 = H // P      # mm2 output chunks

MM_MODE = "bf16"  # "bf16" | "f32r" | "f32"

_nc_cache = {}


def _build(mm_mode):
    f32 = mybir.dt.float32
    bf16 = mybir.dt.bfloat16
    mm_dt = bf16 if mm_mode == "bf16" else f32

    def mm_ap(ap):
        return ap.bitcast(mybir.dt.float32r) if mm_mode == "f32r" else ap

    nc = bacc.Bacc("TRN2", target_bir_lowering=False, debug=False, num_devices=NCORES)

    xT_d = nc.dram_tensor("xT", [IN, BL], mm_dt, kind="ExternalInput")
    hT_d = nc.dram_tensor("hT", [H, BL], f32, kind="ExternalInput")
    w_in_d = nc.dram_tensor("w_in", [IN + H, 2 * H], mm_dt, kind="ExternalInput")
    w_out_d = nc.dram_tensor("w_out", [IN + H, H], mm_dt, kind="ExternalInput")
    b_in_d = nc.dram_tensor("b_in", [P, MO1], f32, kind="ExternalInput")
    b_out_d = nc.dram_tensor("b_out", [P, MO2], f32, kind="ExternalInput")
    h_newT_d = nc.dram_tensor("h_newT", [H, BL], f32, kind="ExternalOutput")
    outT_d = nc.dram_tensor("outT", [H, BL], f32, kind="ExternalOutput")

    AFT = mybir.ActivationFunctionType

    # feature-major DRAM views: row (c*128 + p) <-> (partition p, chunk c)
    x_dram = xT_d.ap().rearrange("(c p) n -> p c n", p=P)
    h_dram = hT_d.ap().rearrange("(c p) n -> p c n", p=P)
    w_in_dram = w_in_d.ap().rearrange("(k p) m -> p k m", p=P)
    hn_dram = h_newT_d.ap().rearrange("(c p) n -> p c n", p=P)
    out_dram = outT_d.ap().rearrange("(c p) n -> p c n", p=P)
    w_out_dram = w_out_d.ap().rearrange("(k p) m -> p k m", p=P)

    with tile.TileContext(nc) as tc, ExitStack() as ctx:
        cpool = ctx.enter_context(tc.tile_pool(name="consts", bufs=1))
        work = ctx.enter_context(tc.tile_pool(name="work", bufs=2))
        tmp_pool = ctx.enter_context(tc.tile_pool(name="tmp", bufs=4))
        psum1 = ctx.enter_context(tc.tile_pool(name="psum1", bufs=4, space="PSUM"))
        psum2 = ctx.enter_context(tc.tile_pool(name="psum2", bufs=4, space="PSUM"))

        # ---- loads, in the order SP should issue them ----
        b_in_sb = cpool.tile([P, MO1], f32, name="b_in_sb")
        nc.sync.dma_start(b_in_sb[:], b_in_d[:])
        b_out_sb = cpool.tile([P, MO2], f32, name="b_out_sb")
        nc.sync.dma_start(b_out_sb[:], b_out_d[:])

        w_in_sb = [cpool.tile([P, 2 * H], mm_dt, name=f"w_in_{k}") for k in range(K1)]
        x_sb = [cpool.tile([P, KIN * NB], mm_dt, name=f"x_sb_{j}") for j in range(NT)]
        h_sb = [cpool.tile([P, KH * NB], f32, name=f"h_sb_{j}") for j in range(NT)]

        def load_xh(j):
            bs = bass.ts(j, NB)
            nc.sync.dma_start(
                x_sb[j][:].rearrange("p (k n) -> p k n", k=KIN), x_dram[:, :, bs]
            )
            nc.sync.dma_start(
                h_sb[j][:].rearrange("p (c n) -> p c n", c=KH), h_dram[:, :, bs]
            )

        for k in range(KIN):
            nc.sync.dma_start(w_in_sb[k][:], w_in_dram[:, k, :])
        load_xh(0)
        for k in range(KIN, K1):
            nc.sync.dma_start(w_in_sb[k][:], w_in_dram[:, k, :])
        load_xh(1)
        w_out_sb = cpool.tile([P, K1 * H], mm_dt, name="w_out_sb")
        nc.sync.dma_start(
            w_out_sb[:].rearrange("p (k m) -> p k m", k=K1), w_out_dram[:]
        )
        load_xh(2)
        load_xh(3)

        for j in range(NT):
            bs = bass.ts(j, NB)

            if mm_mode == "bf16":
                hc = work.tile([P, KH * NB], bf16, name="hc", tag="hc")
                nc.vector.tensor_copy(hc[:], h_sb[j][:])
            else:
                hc = h_sb[j]

            learn = work.tile([P, KH * NB], f32, name="learn", tag="learn")
            forget = work.tile([P, KH * NB], f32, name="forget", tag="forget")
            hn = work.tile([P, KH * NB], f32, name="hn", tag="hn")
            if mm_mode == "bf16":
                hnc = work.tile([P, KH * NB], bf16, name="hnc", tag="hnc")

            # mm1 interleaved (learn_c, forget_c) so h_new chunk c's DVE work
            # overlaps the remaining matmuls instead of serializing at the end
            for c in range(KH):
                for mo in (c, c + KH):
                    ps = psum1.tile([P, NB], f32, name="ps1", tag="ps1")
                    for k in range(K1):
                        rhs = (
                            x_sb[j][:, bass.ts(k, NB)]
                            if k < KIN
                            else hc[:, bass.ts(k - KIN, NB)]
                        )
                        nc.tensor.matmul(
                            ps[:],
                            mm_ap(w_in_sb[k][:, mo * P:(mo + 1) * P]),
                            mm_ap(rhs),
                            start=(k == 0),
                            stop=(k == K1 - 1),
                        )
                    dst = (learn if mo < KH else forget)[:, c * NB:(c + 1) * NB]
                    nc.scalar.activation(
                        dst,
                        ps[:],
                        AFT.Tanh if mo < KH else AFT.Sigmoid,
                        bias=b_in_sb[:, mo:mo + 1],
                    )
                cs = bass.ts(c, NB)
                t = tmp_pool.tile([P, NB], f32, name="t", tag="t")
                nc.vector.tensor_sub(t[:], h_sb[j][:, cs], learn[:, cs])
                nc.vector.tensor_mul(t[:], t[:], forget[:, cs])
                nc.vector.tensor_add(hn[:, cs], t[:], learn[:, cs])
                if mm_mode == "bf16":
                    nc.vector.tensor_copy(hnc[:, cs], hn[:, cs])

            nc.sync.dma_start(
                hn_dram[:, :, bs], hn[:].rearrange("p (c n) -> p c n", c=KH)
            )

            # mm2 k-outer: the x-part (k<KIN) streams while the last h_new
            # chunks are still being produced; hnc chunk c is only needed at
            # stage k = KIN + c.
            hsrc = hnc if mm_mode == "bf16" else hn
            pss = [psum2.tile([P, NB], f32, name="ps2", tag="ps2") for mo in range(MO2)]
            for k in range(K1):
                rhs = (
                    x_sb[j][:, bass.ts(k, NB)]
                    if k < KIN
                    else hsrc[:, bass.ts(k - KIN, NB)]
                )
                for mo in range(MO2):
                    nc.tensor.matmul(
                        pss[mo][:],
                        mm_ap(w_out_sb[:, (k * MO2 + mo) * P:(k * MO2 + mo + 1) * P]),
                        mm_ap(rhs),
                        start=(k == 0),
                        stop=(k == K1 - 1),
                    )
            out_t = work.tile([P, MO2 * NB], f32, name="out_t", tag="out_t")
            for mo in range(MO2):
                nc.scalar.activation(
                    out_t[:, bass.ts(mo, NB)],
                    pss[mo][:],
                    AFT.Tanh,
                    bias=b_out_sb[:, mo:mo + 1],
                )
            half = MO2 // 2
            nc.sync.dma_start(
                out_dram[:, 0:half, bs],
                out_t[:, 0:half * NB].rearrange("p (c n) -> p c n", c=half),
            )
            nc.sync.dma_start(
                out_dram[:, half:MO2, bs],
                out_t[:, half * NB:].rearrange("p (c n) -> p c n", c=MO2 - half),
            )

    nc.compile()
    return nc


def _get_nc(mm_mode):
    if mm_mode not in _nc_cache:
        _nc_cache[mm_mode] = _build(mm_mode)
    return _nc_cache[mm_mode]


def _run(x, h, W_in, b_in, W_out, b_out, mm_mode=MM_MODE, trace=False):
    x = np.asarray(x, dtype=np.float32)
    h = np.asarray(h, dtype=np.float32)
    W_in = np.asarray(W_in, dtype=np.float32)
    b_in = np.asarray(b_in, dtype=np.float32)
    W_out = np.asarray(W_out, dtype=np.float32)
    b_out = np.asarray(b_out, dtype=np.float32)

    bf16 = ml_dtypes.bfloat16
    mm_np = bf16 if mm_mode == "bf16" else np.float32
    w_in_m = np.ascontiguousarray(W_in.astype(mm_np))
    w_out_m = np.ascontiguousarray(W_out.astype(mm_np))
    b_in_m = np.ascontiguousarray(b_in.reshape(MO1, P).T)
    b_out_m = np.ascontiguousarray(b_out.reshape(MO2, P).T)

    in_maps = []
    for i in range(NCORES):
        sl = slice(i * BL, (i + 1) * BL)
        m = {
            "xT": np.ascontiguousarray(x[sl].T).astype(mm_np),
            "hT": np.ascontiguousarray(h[sl].T),
            "w_in": w_in_m,
            "w_out": w_out_m,
            "b_in": b_in_m,
            "b_out": b_out_m,
        }
        in_maps.append(m)

    nc = _get_nc(mm_mode)
    res = run_bass_kernel_spmd(nc, in_maps, list(range(NCORES)), trace=trace)

    out = np.empty((B, H), dtype=np.float32)
    h_new = np.empty((B, H), dtype=np.float32)
    for i in range(NCORES):
        sl = slice(i * BL, (i + 1) * BL)
        out[sl] = res.results[i]["outT"].T
        h_new[sl] = res.results[i]["h_newT"].T
    return (out, h_new), res


def kernel(x, h, W_in, b_in, W_out, b_out):
    (out, h_new), _ = _run(x, h, W_in, b_in, W_out, b_out)
    return (out, h_new)
